# revision 25
# baseline (speedup 1.0000x reference)
"""Trainium2 Bass kernel for nn_Conv2d_86191403696259.

v2d: 2-pass (dh,dw)-folded matmul (256 MMs), fp16 stores via gpsimd SWDGE
(engages all 16 SDMA engines) with 32KB descriptors (8-row / 2-chunk
batching), four groups (G01,G11,G21,G22) DMA'd from host-prepped shifted
streams ("riders"), only 3 on-chip shift copies per chunk, 256B-aligned
input rows.

Pass A tile [128, 2*6WP]: rows 0-23 G(0,0) | 24-31 G22 q0-7 | 32-55 G(1,0)
  | 56-63 G22 q8-15 | 64-87 G(2,0) | 88-95 G22 q16-23 | 96-119 G(0,1) rider.
Pass B tile [128, 2*4WP]: 0-23 G(1,1) rider | 32-55 G(2,1) rider
  | 64-87 G(0,2) | 96-119 G(1,2); gaps zeroed once.
Group (dh,dw) partition q=3j+ic holds x_pad[ic, rs+u+dh+32j, v+dw] at u*WP+v.
Copies: C1 G10<-G00+WP (Act), C2 G20<-G00+2WP (Act),
        C5 B[64:128]<-A[0:64]+2e (DVE).
"""

import numpy as np

from concourse.ap import AP
import concourse.bass as bass
import concourse.mybir as mybir
import concourse.tile as tile
from concourse import bacc
from concourse.bass_utils import run_bass_kernel_spmd

IC, OC, KH, KW = 3, 16, 3, 3
H = W = 2048
N_CORES = 8
RPC = H // N_CORES          # 256
HP = RPC + 2                # 258
WP = W + 2                  # 2050

NB = 8                      # bands
BR = RPC // NB              # 32 rows per band
S = 4                       # rows per chunk
NCHUNK = BR // S            # 8
NWT = W // 512              # 4

PA = 6 * WP                 # pass-A half pitch
PB = 4 * WP                 # pass-B half pitch
L_XS = 12416                # padded xs row (97*128 elems; 24832B = 97*256)
L_XR = 8320                 # padded xr row (65*128 elems; 16640B = 65*256)

F32 = mybir.dt.float32
FP16 = mybir.dt.float16
DT = FP16

# lhsT row maps: (row_start, (dh,dw), q_start, q_count)
MAP_A = [(0, (0, 0), 0, 24), (24, (2, 2), 0, 24), (64, (1, 0), 0, 24),
         (96, (2, 0), 0, 24)]
MAP_B = [(0, (1, 1), 0, 24), (24, (2, 1), 0, 24), (48, (0, 1), 0, 16),
         (64, (0, 2), 0, 24), (88, (0, 1), 16, 8), (96, (1, 2), 0, 24)]


def build_nc() -> bass.Bass:
    nc = bacc.Bacc("TRN2", target_bir_lowering=False, debug=False)
    xs = nc.dram_tensor("xs", [NCHUNK, 24, L_XS], DT, kind="ExternalInput")
    xr = nc.dram_tensor("xr", [NCHUNK, 96, L_XR], DT, kind="ExternalInput")
    xz = nc.dram_tensor("xz", [32, 2 * PA], DT, kind="ExternalInput")
    wa = nc.dram_tensor("wa", [128, 128], DT, kind="ExternalInput")
    wb = nc.dram_tensor("wb", [128, 128], DT, kind="ExternalInput")
    out = nc.dram_tensor("out", [OC, RPC, W], DT, kind="ExternalOutput")
    out_flat = out[:, :, :]

    def store_ap(r0):
        # dims (oc, j, t, w): element = out[oc, 32j + r0 + t, w]
        # oc-major matches stg partition p = 8*oc + j; descriptor addresses
        # step by 128KB (j) instead of 1MB (oc) to avoid HBM thrash.
        return AP(
            out_flat.tensor,
            r0 * W,
            [(RPC * W, OC), (BR * W, NB), (W, 2 * S), (1, W)],
        )

    with tile.TileContext(nc) as tc:
        with (
            tc.tile_pool(name="wpool", bufs=1) as wpool,
            tc.tile_pool(name="slaba", bufs=1) as slaba_pool,
            tc.tile_pool(name="slabb", bufs=1) as slabb_pool,
            tc.tile_pool(name="stgout", bufs=2) as stgout_pool,
            tc.tile_pool(name="psum", bufs=2, space="PSUM") as psum_pool,
        ):
            wa_sb = wpool.tile([128, 128], DT)
            wb_sb = wpool.tile([128, 128], DT)
            nc.sync.dma_start(out=wa_sb[:, :], in_=wa[:, :])
            nc.sync.dma_start(out=wb_sb[:, :], in_=wb[:, :])

            A = slaba_pool.tile([128, 2 * PA], DT)
            B = slabb_pool.tile([128, 2 * PB], DT)
            # zero gap rows + copy-tails once via DMA (parallel with first
            # chunk loads; DVE memset would cost ~34us serial)
            nc.gpsimd.dma_start(out=A[32:64, :], in_=xz[:, :])
            nc.gpsimd.dma_start(out=A[64:96, :], in_=xz[:, :])
            nc.gpsimd.dma_start(out=A[96:128, :], in_=xz[:, :])
            nc.gpsimd.dma_start(out=B[96:128, :], in_=xz[:, 0 : 2 * PB])

            stg = None
            for kc in range(NCHUNK):
                h = kc % 2
                a0 = h * PA
                b0 = h * PB
                # G00: 6 raw rows (24.6KB descriptor per partition)
                nc.sync.dma_start(
                    out=A[0:24, a0 : a0 + 6 * WP], in_=xs[kc, :, 0 : 6 * WP]
                )
                # riders (content pre-shifted on host)
                nc.sync.dma_start(
                    out=A[24:48, a0 : a0 + 4 * WP], in_=xr[kc, 0:24, 0 : 4 * WP]
                )
                nc.sync.dma_start(
                    out=B[0:64, b0 : b0 + 4 * WP], in_=xr[kc, 24:88, 0 : 4 * WP]
                )
                nc.sync.dma_start(
                    out=B[88:96, b0 : b0 + 4 * WP], in_=xr[kc, 88:96, 0 : 4 * WP]
                )
                # shift copies (all DVE: 3.6 elem/ns vs Act's 1 elem/cycle)
                nc.vector.tensor_copy(
                    out=A[64:88, a0 : a0 + 4 * WP],
                    in_=A[0:24, a0 + WP : a0 + 5 * WP],
                )
                nc.vector.tensor_copy(
                    out=A[96:120, a0 : a0 + 4 * WP],
                    in_=A[0:24, a0 + 2 * WP : a0 + 6 * WP],
                )
                nc.vector.tensor_copy(
                    out=B[64:88, b0 : b0 + 4 * WP],
                    in_=A[0:24, a0 + 2 : a0 + 4 * WP + 2],
                )
                nc.vector.tensor_copy(
                    out=B[96:120, b0 : b0 + 4 * WP],
                    in_=A[64:88, a0 + 2 : a0 + 4 * WP + 2],
                )

                if h == 0:
                    stg = stgout_pool.tile([128, 2 * S * W], DT, tag="stg")
                for s in range(S):
                    ps = psum_pool.tile([128, W], F32, tag="ps")
                    for wt in range(NWT):
                        nc.tensor.matmul(
                            out=ps[:, wt * 512 : (wt + 1) * 512],
                            lhsT=wa_sb[:, :],
                            rhs=A[
                                :,
                                a0 + s * WP + wt * 512 : a0
                                + s * WP
                                + wt * 512
                                + 512,
                            ],
                            start=True,
                            stop=False,
                        )
                    for wt in range(NWT):
                        nc.tensor.matmul(
                            out=ps[:, wt * 512 : (wt + 1) * 512],
                            lhsT=wb_sb[:, :],
                            rhs=B[
                                :,
                                b0 + s * WP + wt * 512 : b0
                                + s * WP
                                + wt * 512
                                + 512,
                            ],
                            start=False,
                            stop=True,
                        )
                    slot = (h * S + s) * W
                    nc.scalar.copy(out=stg[:, slot : slot + W], in_=ps[:, :])
                if h == 1:
                    nc.gpsimd.dma_start(
                        out=store_ap(S * (kc - 1)), in_=stg[:, :]
                    )

    nc.compile()
    return nc


def make_weights(kernel: np.ndarray):
    """kernel [OC, IC, KH, KW] -> lhsT [128,128] for passes A and B."""
    def mk(rowmap):
        wd = np.zeros((128, 128), np.float32)
        ocs = np.arange(OC)
        for r0, (dh, dw), q0, qn in rowmap:
            for qq in range(qn):
                j, ic = divmod(q0 + qq, 3)
                wd[r0 + qq, 8 * ocs + j] = kernel[:, ic, dh, dw]
        return np.ascontiguousarray(wd.astype(np.float16))

    return mk(MAP_A), mk(MAP_B)


def _rows(x_pad16, c, dh, nrow):
    """[NCHUNK, NB, IC, nrow, WP] fp16: x_pad[ic, c*RPC + 4kc + 32j + u + dh]."""
    xsl = x_pad16[:, c * RPC : c * RPC + HP, :]
    kcs = np.arange(NCHUNK)[:, None, None]
    js = np.arange(NB)[None, :, None]
    us = np.arange(nrow)[None, None, :]
    rows = S * kcs + BR * js + us + dh
    g = xsl[:, rows, :]                               # [IC, kc, j, nrow, WP]
    return g.transpose(1, 2, 0, 3, 4)                 # [kc, j, ic, nrow, WP]


def make_xs(x_pad16: np.ndarray, c: int) -> np.ndarray:
    """xs[kc, 3j+ic, u*WP+v] = x_pad[ic, rs+u+32j, v], u in [0,6); padded rows."""
    g = _rows(x_pad16, c, 0, 6).reshape(NCHUNK, 24, 6 * WP)
    outb = np.zeros((NCHUNK, 24, L_XS), np.float16)
    outb[:, :, : 6 * WP] = g
    return np.ascontiguousarray(outb)


def make_xr(x_pad16: np.ndarray, c: int) -> np.ndarray:
    """Riders: rows 0-23 G(2,2), 24-47 G(1,1), 48-71 G(2,1), 72-95 G(0,1).
    Group (dh,dw) row q=3j+ic slot u holds x_pad[ic, rs+u+dh+32j, v+dw]."""
    outb = np.zeros((NCHUNK, 96, L_XR), np.float16)
    for gi, (dh, dw) in enumerate([(2, 2), (1, 1), (2, 1), (0, 1)]):
        g = _rows(x_pad16, c, dh, 4)                  # [kc, j, ic, 4, WP]
        blk = np.zeros((NCHUNK, NB, IC, 4, WP), np.float16)
        blk[:, :, :, :, : WP - dw] = g[:, :, :, :, dw:]
        outb[:, 24 * gi : 24 * gi + 24, : 4 * WP] = blk.reshape(
            NCHUNK, 24, 4 * WP
        )
    return np.ascontiguousarray(outb)


_NC_CACHE = {}


def kernel(x: np.ndarray, kernel: np.ndarray) -> np.ndarray:
    assert x.shape == (IC, H, W) and kernel.shape == (OC, IC, KH, KW)
    x = np.ascontiguousarray(x, np.float32)
    kernel = np.ascontiguousarray(kernel, np.float32)

    if "nc" not in _NC_CACHE:
        _NC_CACHE["nc"] = build_nc()
    nc = _NC_CACHE["nc"]

    x_pad = np.zeros((IC, H + 2, W + 2), np.float16)
    x_pad[:, 1:-1, 1:-1] = x.astype(np.float16)
    wa, wb = make_weights(kernel)

    xz = np.zeros((32, 2 * PA), np.float16)
    in_maps = []
    for c in range(N_CORES):
        in_maps.append(
            {"xs": make_xs(x_pad, c), "xr": make_xr(x_pad, c), "wa": wa,
             "wb": wb, "xz": xz}
        )

    res = run_bass_kernel_spmd(nc, in_maps, core_ids=list(range(N_CORES)))
    outs = [res.results[c]["out"].astype(np.float32) for c in range(N_CORES)]
    return np.concatenate(outs, axis=1)


# revision 27
# speedup vs baseline: 1.0898x; 1.0898x over previous
"""Trainium2 Bass kernel for nn_Conv2d_86191403696259.

v2d: 2-pass (dh,dw)-folded matmul (256 MMs), fp16 stores via gpsimd SWDGE
(engages all 16 SDMA engines) with 32KB descriptors (8-row / 2-chunk
batching), four groups (G01,G11,G21,G22) DMA'd from host-prepped shifted
streams ("riders"), only 3 on-chip shift copies per chunk, 256B-aligned
input rows.

Pass A tile [128, 2*6WP]: rows 0-23 G(0,0) | 24-31 G22 q0-7 | 32-55 G(1,0)
  | 56-63 G22 q8-15 | 64-87 G(2,0) | 88-95 G22 q16-23 | 96-119 G(0,1) rider.
Pass B tile [128, 2*4WP]: 0-23 G(1,1) rider | 32-55 G(2,1) rider
  | 64-87 G(0,2) | 96-119 G(1,2); gaps zeroed once.
Group (dh,dw) partition q=3j+ic holds x_pad[ic, rs+u+dh+32j, v+dw] at u*WP+v.
Copies: C1 G10<-G00+WP (Act), C2 G20<-G00+2WP (Act),
        C5 B[64:128]<-A[0:64]+2e (DVE).
"""

import numpy as np

from concourse.ap import AP
import concourse.bass as bass
import concourse.mybir as mybir
import concourse.tile as tile
from concourse import bacc
from concourse.bass_utils import run_bass_kernel_spmd

IC, OC, KH, KW = 3, 16, 3, 3
H = W = 2048
N_CORES = 8
RPC = H // N_CORES          # 256
HP = RPC + 2                # 258
WP = W + 2                  # 2050

NB = 8                      # bands
BR = RPC // NB              # 32 rows per band
S = 4                       # rows per chunk
NCHUNK = BR // S            # 8
NWT = W // 512              # 4

PA = 6 * WP                 # pass-A half pitch
PB = 4 * WP                 # pass-B half pitch
L_XS = 12416                # padded xs row (97*128 elems; 24832B = 97*256)
L_XR = 8320                 # padded xr row (65*128 elems; 16640B = 65*256)

F32 = mybir.dt.float32
FP16 = mybir.dt.float16
DT = FP16

# lhsT row maps: (row_start, (dh,dw), q_start, q_count)
MAP_A = [(0, (0, 0), 0, 24), (24, (2, 2), 0, 24), (64, (1, 0), 0, 24),
         (96, (2, 0), 0, 24)]
MAP_B = [(0, (1, 1), 0, 24), (24, (2, 1), 0, 24), (48, (0, 1), 0, 16),
         (64, (0, 2), 0, 24), (88, (0, 1), 16, 8), (96, (1, 2), 0, 24)]


def build_nc() -> bass.Bass:
    nc = bacc.Bacc("TRN2", target_bir_lowering=False, debug=False)
    xs = nc.dram_tensor("xs", [NCHUNK, 24, L_XS], DT, kind="ExternalInput")
    xr = nc.dram_tensor("xr", [NCHUNK, 96, L_XR], DT, kind="ExternalInput")
    xz = nc.dram_tensor("xz", [32, 2 * PA], DT, kind="ExternalInput")
    wa = nc.dram_tensor("wa", [128, 128], DT, kind="ExternalInput")
    wb = nc.dram_tensor("wb", [128, 128], DT, kind="ExternalInput")
    out = nc.dram_tensor("out", [OC, RPC, W], DT, kind="ExternalOutput")
    out_flat = out[:, :, :]

    def store_ap(r0):
        # dims (oc, j, t, w): element = out[oc, 32j + r0 + t, w]
        # oc-major matches stg partition p = 8*oc + j; descriptor addresses
        # step by 128KB (j) instead of 1MB (oc) to avoid HBM thrash.
        return AP(
            out_flat.tensor,
            r0 * W,
            [(RPC * W, OC), (BR * W, NB), (W, 2 * S), (1, W)],
        )

    with tile.TileContext(nc) as tc:
        with (
            tc.tile_pool(name="wpool", bufs=1) as wpool,
            tc.tile_pool(name="slaba", bufs=1) as slaba_pool,
            tc.tile_pool(name="slabb", bufs=1) as slabb_pool,
            tc.tile_pool(name="stgout", bufs=2) as stgout_pool,
            tc.tile_pool(name="psum", bufs=2, space="PSUM") as psum_pool,
        ):
            wa_sb = wpool.tile([128, 128], DT)
            wb_sb = wpool.tile([128, 128], DT)
            nc.sync.dma_start(out=wa_sb[:, :], in_=wa[:, :])
            nc.sync.dma_start(out=wb_sb[:, :], in_=wb[:, :])

            A = slaba_pool.tile([128, 2 * PA], DT)
            B = slabb_pool.tile([128, 2 * PB], DT)
            # zero gap rows + copy-tails once via DMA (parallel with first
            # chunk loads; DVE memset would cost ~34us serial)
            nc.scalar.dma_start(out=A[32:64, :], in_=xz[:, :])
            nc.scalar.dma_start(out=A[64:96, :], in_=xz[:, :])
            nc.scalar.dma_start(out=A[96:128, :], in_=xz[:, :])
            nc.scalar.dma_start(out=B[96:128, :], in_=xz[:, 0 : 2 * PB])

            stg = None
            for kc in range(NCHUNK):
                h = kc % 2
                a0 = h * PA
                b0 = h * PB
                # G00: 6 raw rows (24.6KB descriptor per partition)
                nc.sync.dma_start(
                    out=A[0:24, a0 : a0 + 6 * WP], in_=xs[kc, :, 0 : 6 * WP]
                )
                # riders (content pre-shifted on host)
                nc.sync.dma_start(
                    out=A[24:48, a0 : a0 + 4 * WP], in_=xr[kc, 0:24, 0 : 4 * WP]
                )
                nc.scalar.dma_start(
                    out=B[0:64, b0 : b0 + 4 * WP], in_=xr[kc, 24:88, 0 : 4 * WP]
                )
                nc.scalar.dma_start(
                    out=B[88:96, b0 : b0 + 4 * WP], in_=xr[kc, 88:96, 0 : 4 * WP]
                )
                # shift copies (all DVE: 3.6 elem/ns vs Act's 1 elem/cycle)
                nc.vector.tensor_copy(
                    out=A[64:88, a0 : a0 + 4 * WP],
                    in_=A[0:24, a0 + WP : a0 + 5 * WP],
                )
                nc.vector.tensor_copy(
                    out=A[96:120, a0 : a0 + 4 * WP],
                    in_=A[0:24, a0 + 2 * WP : a0 + 6 * WP],
                )
                nc.vector.tensor_copy(
                    out=B[64:88, b0 : b0 + 4 * WP],
                    in_=A[0:24, a0 + 2 : a0 + 4 * WP + 2],
                )
                nc.vector.tensor_copy(
                    out=B[96:120, b0 : b0 + 4 * WP],
                    in_=A[64:88, a0 + 2 : a0 + 4 * WP + 2],
                )

                if h == 0:
                    stg = stgout_pool.tile([128, 2 * S * W], DT, tag="stg")
                for s in range(S):
                    ps = psum_pool.tile([128, W], F32, tag="ps")
                    for wt in range(NWT):
                        nc.tensor.matmul(
                            out=ps[:, wt * 512 : (wt + 1) * 512],
                            lhsT=wa_sb[:, :],
                            rhs=A[
                                :,
                                a0 + s * WP + wt * 512 : a0
                                + s * WP
                                + wt * 512
                                + 512,
                            ],
                            start=True,
                            stop=False,
                        )
                    for wt in range(NWT):
                        nc.tensor.matmul(
                            out=ps[:, wt * 512 : (wt + 1) * 512],
                            lhsT=wb_sb[:, :],
                            rhs=B[
                                :,
                                b0 + s * WP + wt * 512 : b0
                                + s * WP
                                + wt * 512
                                + 512,
                            ],
                            start=False,
                            stop=True,
                        )
                    slot = (h * S + s) * W
                    nc.scalar.copy(out=stg[:, slot : slot + W], in_=ps[:, :])
                if h == 1:
                    nc.gpsimd.dma_start(
                        out=store_ap(S * (kc - 1)), in_=stg[:, :]
                    )

    nc.compile()
    return nc


def make_weights(kernel: np.ndarray):
    """kernel [OC, IC, KH, KW] -> lhsT [128,128] for passes A and B."""
    def mk(rowmap):
        wd = np.zeros((128, 128), np.float32)
        ocs = np.arange(OC)
        for r0, (dh, dw), q0, qn in rowmap:
            for qq in range(qn):
                j, ic = divmod(q0 + qq, 3)
                wd[r0 + qq, 8 * ocs + j] = kernel[:, ic, dh, dw]
        return np.ascontiguousarray(wd.astype(np.float16))

    return mk(MAP_A), mk(MAP_B)


def _rows(x_pad16, c, dh, nrow):
    """[NCHUNK, NB, IC, nrow, WP] fp16: x_pad[ic, c*RPC + 4kc + 32j + u + dh]."""
    xsl = x_pad16[:, c * RPC : c * RPC + HP, :]
    kcs = np.arange(NCHUNK)[:, None, None]
    js = np.arange(NB)[None, :, None]
    us = np.arange(nrow)[None, None, :]
    rows = S * kcs + BR * js + us + dh
    g = xsl[:, rows, :]                               # [IC, kc, j, nrow, WP]
    return g.transpose(1, 2, 0, 3, 4)                 # [kc, j, ic, nrow, WP]


def make_xs(x_pad16: np.ndarray, c: int) -> np.ndarray:
    """xs[kc, 3j+ic, u*WP+v] = x_pad[ic, rs+u+32j, v], u in [0,6); padded rows."""
    g = _rows(x_pad16, c, 0, 6).reshape(NCHUNK, 24, 6 * WP)
    outb = np.zeros((NCHUNK, 24, L_XS), np.float16)
    outb[:, :, : 6 * WP] = g
    return np.ascontiguousarray(outb)


def make_xr(x_pad16: np.ndarray, c: int) -> np.ndarray:
    """Riders: rows 0-23 G(2,2), 24-47 G(1,1), 48-71 G(2,1), 72-95 G(0,1).
    Group (dh,dw) row q=3j+ic slot u holds x_pad[ic, rs+u+dh+32j, v+dw]."""
    outb = np.zeros((NCHUNK, 96, L_XR), np.float16)
    for gi, (dh, dw) in enumerate([(2, 2), (1, 1), (2, 1), (0, 1)]):
        g = _rows(x_pad16, c, dh, 4)                  # [kc, j, ic, 4, WP]
        blk = np.zeros((NCHUNK, NB, IC, 4, WP), np.float16)
        blk[:, :, :, :, : WP - dw] = g[:, :, :, :, dw:]
        outb[:, 24 * gi : 24 * gi + 24, : 4 * WP] = blk.reshape(
            NCHUNK, 24, 4 * WP
        )
    return np.ascontiguousarray(outb)


_NC_CACHE = {}


def kernel(x: np.ndarray, kernel: np.ndarray) -> np.ndarray:
    assert x.shape == (IC, H, W) and kernel.shape == (OC, IC, KH, KW)
    x = np.ascontiguousarray(x, np.float32)
    kernel = np.ascontiguousarray(kernel, np.float32)

    if "nc" not in _NC_CACHE:
        _NC_CACHE["nc"] = build_nc()
    nc = _NC_CACHE["nc"]

    x_pad = np.zeros((IC, H + 2, W + 2), np.float16)
    x_pad[:, 1:-1, 1:-1] = x.astype(np.float16)
    wa, wb = make_weights(kernel)

    xz = np.zeros((32, 2 * PA), np.float16)
    in_maps = []
    for c in range(N_CORES):
        in_maps.append(
            {"xs": make_xs(x_pad, c), "xr": make_xr(x_pad, c), "wa": wa,
             "wb": wb, "xz": xz}
        )

    res = run_bass_kernel_spmd(nc, in_maps, core_ids=list(range(N_CORES)))
    outs = [res.results[c]["out"].astype(np.float32) for c in range(N_CORES)]
    return np.concatenate(outs, axis=1)
